# revision 43
# baseline (speedup 1.0000x reference)
"""Trainium2 Bass kernel for nn_BasicTransDecoderBlock (dense_transformer).

Strategy: data-parallel over batch B=8 across 8 NeuronCores (1 sample/core).
v2: BatchNorm is reassociated THROUGH the depthwise convs (DW(S*x+T) =
S*DW(x) + T*M with the border field M handled exactly as 9 rank-1 terms
folded into the pointwise matmul as extra contraction rows).  This lets all
depthwise tap work start immediately on the raw inputs and fully overlap
the BN AllReduce.  Taps run as tensor_scalar mult (4x mode) + tensor_tensor
add (2x mode) instead of 1x scalar_tensor_tensor; odd-dx tap multiplies go
to the scalar engine.  BN sums ride free on activation accum_out.  The
attention is softmax-free and reassociated: O = Q'(K'^T V')/d + (bias V')/d,
with the bias term folded into the attention matmul via a transposed-BV
(BVCT) extra contraction and per-head LN-q gain folded into the SELB
broadcast selector.

Self-contained: hardcodes all shapes; imports only the concourse runtime
shipped in the container.
"""
import sys
import numpy as np
import ml_dtypes

for _p in ("/opt/trn_rl_repo", "/root/.axon_site/_ro/trn_rl_repo"):
    if _p not in sys.path:
        sys.path.insert(0, _p)

import concourse.bass as bass
import concourse.bacc as bacc
import concourse.tile as tile
from concourse import mybir
from concourse.bass_utils import run_bass_kernel_spmd

FP32 = mybir.dt.float32
BF16 = mybir.dt.bfloat16
ALU = mybir.AluOpType
ACTF = mybir.ActivationFunctionType

B, IN_CH, OUT_CH, HEADS, DIM_HEAD, R = 8, 512, 256, 8, 32, 16
H1, W1, H2, W2 = 32, 32, 64, 64
EPS_BN, EPS_LN = 1e-5, 1e-6
N1, N2, NS = H1 * W1, H2 * W2, R * R     # 1024, 4096, 256
P = 128
NCORES = 8
PW1, PW2 = W1 + 2, W2 + 2                # padded widths 34, 66
PAD1, PAD2 = (H1 + 2) * PW1, (H2 + 2) * PW2   # 1156, 4356
TAPS = [(dy, dx) for dy in range(3) for dx in range(3)]


# ---------------------------------------------------------------- host helpers

def _interp_matrix(n_in, n_out):
    A = np.zeros((n_out, n_in), np.float32)
    xs = np.linspace(0.0, n_in - 1.0, n_out)
    for i, x in enumerate(xs):
        x0 = int(np.floor(x)); x1 = min(x0 + 1, n_in - 1)
        w = x - x0
        A[i, x0] += 1.0 - w
        A[i, x1] += w
    return A


def _head_major_perm():
    perm = np.zeros(OUT_CH, np.int64)
    for h in range(HEADS):
        for d in range(DIM_HEAD):
            perm[h * DIM_HEAD + d] = d * HEADS + h
    return perm


def _rel_bias_small(rel_table):
    c = np.stack(np.meshgrid(np.arange(R), np.arange(R), indexing="ij")).reshape(2, -1)
    rel = (c[:, :, None] - c[:, None, :]).transpose(1, 2, 0)
    rel[:, :, 0] += R - 1
    rel[:, :, 1] += R - 1
    rel[:, :, 0] *= 2 * R - 1
    idx = rel.sum(-1).reshape(-1)
    return np.asarray(rel_table, np.float32)[idx].reshape(NS, NS, HEADS)


def _r64_chunks():
    """Residue resize (32->64), ch-major: per 512-pixel output chunk only a
    few 128-pixel input tiles contribute."""
    Ay, Ax = _interp_matrix(H1, H2), _interp_matrix(W1, W2)
    R64 = np.kron(Ay, Ax).astype(np.float32)       # [4096, 1024]
    ktiles, blocks = [], []
    for nn in range(8):
        rows = R64[nn * 512:(nn + 1) * 512]
        used = [kk for kk in range(8)
                if np.abs(rows[:, kk * 128:(kk + 1) * 128]).sum() > 0]
        ktiles.append(used)
        for kk in used:
            blocks.append(rows[:, kk * 128:(kk + 1) * 128].T.copy())
    return ktiles, np.concatenate(blocks, axis=0)


_R64_KTILES, _R64_PACKED = _r64_chunks()
_N_R64_SLOTS = sum(len(k) for k in _R64_KTILES)


def _valid_field(H, W, rows):
    """[9, 3, rows*W] tap-validity band patterns: 0=top rows, 1=middle,
    2=bottom rows (chunks repeat the middle pattern)."""
    v = np.zeros((9, 3, rows * W), np.float32)
    for b, y0 in enumerate((0, rows, H - rows)):
        ys, xs = np.mgrid[y0:y0 + rows, 0:W]
        for i, (dy, dx) in enumerate(TAPS):
            ok = ((ys + dy - 1 >= 0) & (ys + dy - 1 < H)
                  & (xs + dx - 1 >= 0) & (xs + dx - 1 < W))
            v[i, b] = ok.reshape(-1)
    return v.reshape(9, 3 * rows * W)


def _host_prep(inp):
    perm = _head_major_perm()
    f32 = lambda a: np.ascontiguousarray(np.asarray(a, np.float32))
    bf = lambda a: np.ascontiguousarray(np.asarray(a, np.float32).astype(ml_dtypes.bfloat16))

    kvw = np.asarray(inp["to_kv_pw"], np.float32).reshape(2 * OUT_CH, IN_CH)
    gq_flat = np.asarray(inp["normq_g"], np.float32).reshape(OUT_CH)
    selb = np.zeros((16 * P, OUT_CH), np.float32)
    hh = np.arange(OUT_CH) // DIM_HEAD
    for blk in range(16):
        for h in range(HEADS):
            # stats row packing (set by the relayout DMA stream order):
            # row = 16*(blk//2) + 2h + (blk%2)
            selb[blk * P + 16 * (blk // 2) + 2 * h + (blk % 2), :] = \
                (hh == h) * gq_flat
    xsel = np.zeros((R, NS), np.float32)      # [xs, yr*64+x] = (x//4 == xs)
    for x in range(W2):
        for yr in range(4):
            xsel[x // 4, yr * W2 + x] = 1.0
    d = {
        "wch": bf(np.asarray(inp["conv_ch_w"], np.float32).reshape(OUT_CH, IN_CH).T),
        "wkv": bf(np.concatenate([kvw[perm].T, kvw[OUT_CH + perm].T], axis=1)),
        "wq": bf(np.asarray(inp["to_q_pw"], np.float32).reshape(OUT_CH, OUT_CH)[perm].T),
        "wout": bf(np.asarray(inp["to_out_pw"], np.float32).reshape(OUT_CH, OUT_CH)[:, perm].T),
        "wmlp": bf(np.asarray(inp["mlp_w"], np.float32).reshape(OUT_CH, OUT_CH).T),
        "dw1w": f32(np.asarray(inp["to_kv_dw"], np.float32).reshape(IN_CH, 9)),
        "dwqw": f32(np.asarray(inp["to_q_dw"], np.float32).reshape(OUT_CH, 9)),
        "dwow": f32(np.asarray(inp["to_out_dw"], np.float32).reshape(OUT_CH, 9)[perm]),
        "rt16": bf(np.kron(_interp_matrix(H1, R), _interp_matrix(W1, R)).T),
        "r64c": bf(_R64_PACKED),
        "selq": bf(np.equal(np.arange(OUT_CH)[:, None] // DIM_HEAD,
                            np.arange(HEADS)[None, :]).astype(np.float32)),
        "selb": bf(selb),
        "bvt": bf(_rel_bias_small(inp["rel_table"]).transpose(2, 1, 0)
                  .reshape(HEADS * NS, NS)),
        "validk": bf(_valid_field(H1, W1, 4)),     # 128-pixel chunks
        "validq": bf(_valid_field(H2, W2, 8)),     # 512-pixel chunks
        "xsel": bf(xsel),
        "gkb": bf(np.tile(np.asarray(inp["normk_g"], np.float32).reshape(1, OUT_CH), (P, 1))),
        "bkb": bf(np.tile(np.asarray(inp["normk_b"], np.float32).reshape(1, OUT_CH), (P, 1))),
        "bq": f32(np.asarray(inp["normq_b"], np.float32).reshape(OUT_CH, 1)),
    }
    dwq9 = np.asarray(inp["to_q_dw"], np.float32).reshape(OUT_CH, 9)
    pd = np.zeros((2 * 9 * P, P), np.float32)
    for t in range(2):
        for i in range(9):
            pd[(t * 9 + i) * P:(t * 9 + i + 1) * P, :] = \
                np.diag(dwq9[P * t:P * t + P, i])
    d["pdiagq"] = bf(pd)
    dwo9 = np.asarray(inp["to_out_dw"], np.float32).reshape(OUT_CH, 9)[perm]
    pdo = np.zeros((2 * 9 * P, P), np.float32)
    for t in range(2):
        for i in range(9):
            pdo[(t * 9 + i) * P:(t * 9 + i + 1) * P, :] = \
                np.diag(dwo9[P * t:P * t + P, i])
    d["pdiago"] = bf(pdo)
    pk = np.zeros((P, 18), np.float32)
    pk[:, 0:4] = np.asarray(inp["norm_l_g"], np.float32).reshape(4, P).T
    pk[:, 4:8] = np.asarray(inp["norm_l_b"], np.float32).reshape(4, P).T
    pk[:, 8:10] = np.asarray(inp["norm_h_g"], np.float32).reshape(2, P).T
    pk[:, 10:12] = np.asarray(inp["norm_h_b"], np.float32).reshape(2, P).T
    pk[:, 12:14] = np.asarray(inp["norm2_g"], np.float32).reshape(2, P).T
    pk[:, 14:16] = np.asarray(inp["norm2_b"], np.float32).reshape(2, P).T
    pk[:, 16:18] = np.asarray(inp["conv_ch_b"], np.float32).reshape(2, P).T
    d["bnpk"] = pk
    return d


# ---------------------------------------------------------------- device build

def _img(ap, w):
    return ap.rearrange("p (h w) -> p h w", w=w)


def _emit_borders(nc, xpad, Hs, pw):
    """zero the 1-px pad frame of xpad [p, (Hs+2)*pw]."""
    v = _img(xpad, pw)
    nc.gpsimd.memset(v[:, 0:1, :], 0.0)
    nc.gpsimd.memset(v[:, Hs + 1:Hs + 2, :], 0.0)
    nc.gpsimd.memset(v[:, 1:Hs + 1, 0:1], 0.0)
    nc.gpsimd.memset(v[:, 1:Hs + 1, pw - 2:pw], 0.0)


def _emit_dw(nc, tmppool, out, xpad, wvec, Hs, Ws, pw, act_odd=True):
    """depthwise 3x3 pad1 on raw input: out [p, Hs*Ws] bf16,
    xpad [p, (Hs+2)*pw] bf16, wvec [p, 9] fp32.
    tap (0,1) initializes dst on scalar engine; even-dx mults on vector
    (tensor_scalar, 4x), odd-dx mults on scalar engine; adds on vector
    (tensor_tensor, 2x)."""
    dst = _img(out, Ws)
    xv = _img(xpad, pw)
    srcs = {(dy, dx): xv[:, dy:dy + Hs, dx:dx + Ws] for dy, dx in TAPS}
    i01 = TAPS.index((0, 1))
    nc.scalar.activation(dst, srcs[(0, 1)], ACTF.Copy, scale=wvec[:, i01:i01 + 1])
    for i, (dy, dx) in enumerate(TAPS):
        if (dy, dx) == (0, 1):
            continue
        tmp = tmppool.tile([P, Hs * Ws], BF16, tag="tmps")
        tv = _img(tmp[:], Ws)
        if dx == 1 and act_odd:
            nc.scalar.activation(tv, srcs[(dy, dx)], ACTF.Copy,
                                 scale=wvec[:, i:i + 1])
        else:
            nc.vector.tensor_scalar(tv, srcs[(dy, dx)], wvec[:, i:i + 1],
                                    None, ALU.mult)
        nc.vector.tensor_add(out, out, tmp[:])


def _emit(nc, tc, dram, out_d):
    import contextlib
    ctx = contextlib.ExitStack()
    pool = lambda name, bufs, space="SBUF": ctx.enter_context(
        tc.tile_pool(name=name, bufs=bufs, space=space))

    consts = pool("consts", 1)
    work = pool("work", 1)        # unique-tag persistents (small)
    pb32 = pool("pb32", 1)        # 32KB class: X2 fp32 -> OSB fp32
    pbA = pool("pbA", 2)          # 17.4KB class: X2BP, OPAD
    pbB = pool("pbB", 2)          # 16KB class: X1fp32/scratch/DWQ/Q/DWO/RELU
    tmps = pool("tmps", 2)        # tap temporaries (8KB)
    p8 = pool("p8", 1)            # 8KB class: DW1 -> BVCT8
    dgp = pool("dgp", 2)          # streamed diag stationaries (256B)
    ps = pool("ps", 4, "PSUM")
    pss = pool("pss", 2, "PSUM")
    dpool = pool("dramp", 1, "DRAM")

    dma = nc.sync.dma_start

    # ---------------- raw inputs: plain fp32 (stats) + padded bf16 (taps)
    X1B = work.tile([P, 4, N1], BF16, tag="X1B")
    dma(X1B[:], dram["x1"].ap().rearrange("(t p) n -> p t n", p=P))
    X2F = pb32.tile([P, 2, N2], FP32, tag="pb32")
    dma(X2F[:], dram["x2"].ap().rearrange("(t p) n -> p t n", p=P))

    def load_c(name, shape, dt=FP32):
        t = consts.tile(shape, dt, tag=name)
        src = dram[name].ap()
        if len(shape) == 3:
            src = src.rearrange("(t p) n -> p t n", p=shape[0])
        dma(t[:], src)
        return t

    WCH = load_c("wch", [P, 4, OUT_CH], BF16)
    WKV = load_c("wkv", [P, 4, 2 * OUT_CH], BF16)
    WQ = load_c("wq", [P, 2, OUT_CH], BF16)
    WOUT = load_c("wout", [P, 2, OUT_CH], BF16)
    WMLP = load_c("wmlp", [P, 2, OUT_CH], BF16)
    DW1W = load_c("dw1w", [P, 4, 9])
    DWQW = load_c("dwqw", [P, 2, 9])
    DWOW = load_c("dwow", [P, 2, 9])
    RT16 = load_c("rt16", [P, 8, NS], BF16)
    R64C = load_c("r64c", [P, _N_R64_SLOTS, 512], BF16)
    SELQ = load_c("selq", [P, 2, HEADS], BF16)
    SELB = load_c("selb", [P, 16, OUT_CH], BF16)
    BVT = load_c("bvt", [P, 2 * HEADS, NS], BF16)
    VALK = load_c("validk", [9, 3, 4 * W1], BF16)
    VALQ = load_c("validq", [9, 3, 8 * W2], BF16)
    XSEL = load_c("xsel", [R, NS], BF16)
    GKB = load_c("gkb", [P, OUT_CH], BF16)
    BKB = load_c("bkb", [P, OUT_CH], BF16)
    BQ = load_c("bq", [P, 2, 1])
    BNPK = load_c("bnpk", [P, 18])

    # padded bf16 copies (scalar engine) + BN1 stats on vector bn_stats
    ccin = work.tile([P, 12], FP32, tag="ccin")
    X1BP = work.tile([P, 4, PAD1], BF16, tag="X1BP")
    for t in range(4):
        _emit_borders(nc, X1BP[:, t, :], H1, PW1)
        nc.scalar.copy(_img(X1BP[:, t, :], PW1)[:, 1:1 + H1, 1:1 + W1],
                       _img(X1B[:, t, :], W1))
    GD = 66                                     # flat guard (>= W2+1)
    X2BG = pbA.tile([P, 2, N2 + 2 * GD], BF16, tag="pbA")
    for t in range(2):
        nc.gpsimd.memset(X2BG[:, t, 0:GD], 0.0)
        nc.gpsimd.memset(X2BG[:, t, GD + N2:], 0.0)
        nc.scalar.copy(X2BG[:, t, GD:GD + N2], X2F[:, t, :])
    statA = work.tile([P, 4, 12], FP32, tag="statA")
    aggA = work.tile([P, 4, 2], FP32, tag="aggA")
    stat2 = work.tile([P, 2, 48], FP32, tag="stat2")
    agg2 = work.tile([P, 2, 2], FP32, tag="agg2")
    for t in range(4):
        for c in range(2):
            nc.vector.bn_stats(statA[:, t, 6 * c:6 * c + 6], X1B[:, t, bass.ts(c, 512)])
        nc.vector.bn_aggr(aggA[:, t, :],
                          statA[:, t, :].rearrange("p (c s) -> p c s", s=6))
    for t in range(2):
        for c in range(8):
            nc.vector.bn_stats(stat2[:, t, 6 * c:6 * c + 6], X2F[:, t, bass.ts(c, 512)])
        nc.vector.bn_aggr(agg2[:, t, :],
                          stat2[:, t, :].rearrange("p (c s) -> p c s", s=6))
    for t in range(6):
        n = float(N1 if t < 4 else N2)
        ag = aggA[:, t, :] if t < 4 else agg2[:, t - 4, :]
        m = ag[:, 0:1]; v = ag[:, 1:2]
        S, S2 = ccin[:, 2 * t:2 * t + 1], ccin[:, 2 * t + 1:2 * t + 2]
        nc.vector.tensor_scalar(S, m, n, None, ALU.mult)
        nc.vector.tensor_mul(S2, m, m)
        nc.vector.tensor_add(S2, S2, v)
        nc.vector.tensor_scalar(S2, S2, n, None, ALU.mult)

    # ---------------- depthwise on RAW inputs (overlaps the AllReduce)
    DW1 = p8.tile([P, 4, N1], BF16, tag="p8")
    for t in range(4):
        _emit_dw(nc, tmps, DW1[:, t, :], X1BP[:, t, :], DW1W[:, t, :], H1, W1, PW1)
    KVT = pbB.tile([P, 8, 2 * OUT_CH], BF16, tag="pbB")
    DWQ = pbB.tile([P, 2, N2], BF16, tag="pbB")

    def _dwq_colfix(t):
        ccol = tmps.tile([P, 2, H2], FP32, tag="tmps")
        nc.vector.memset(ccol[:], 0.0)
        xim = _img(X2F[:, t, :], W2)
        for dy in range(3):
            y0, y1 = max(0, 2 - dy), min(H2, H2 + 2 - dy)
            nc.vector.scalar_tensor_tensor(
                ccol[:, 0, y0:y1].unsqueeze(2),
                xim[:, y0 + dy - 2:y1 + dy - 2, W2 - 1:W2],
                DWQW[:, t, 3 * dy:3 * dy + 1], ccol[:, 0, y0:y1].unsqueeze(2),
                ALU.mult, ALU.add)
            z1 = H2 - dy
            nc.vector.scalar_tensor_tensor(
                ccol[:, 1, 0:z1].unsqueeze(2), xim[:, dy:z1 + dy, 0:1],
                DWQW[:, t, 3 * dy + 2:3 * dy + 3], ccol[:, 1, 0:z1].unsqueeze(2),
                ALU.mult, ALU.add)
        dwim = _img(DWQ[:, t, :], W2)
        nc.vector.scalar_tensor_tensor(dwim[:, :, 0:1], ccol[:, 0, :].unsqueeze(2),
                                       -1.0, dwim[:, :, 0:1], ALU.mult, ALU.add)
        nc.vector.scalar_tensor_tensor(dwim[:, :, W2 - 1:W2],
                                       ccol[:, 1, :].unsqueeze(2),
                                       -1.0, dwim[:, :, W2 - 1:W2], ALU.mult, ALU.add)

    # tile 1: guarded-flat taps on vector+scalar (parallel with tile 0 on PE)
    tq = 1
    iq01 = TAPS.index((0, 1))
    nc.vector.tensor_scalar(DWQ[:, tq, :],
                            X2BG[:, tq, bass.ds(GD - W2, N2)],
                            DWQW[:, tq, iq01:iq01 + 1], None, ALU.mult)
    for i, (dy, dx) in enumerate(TAPS):
        if (dy, dx) == (0, 1):
            continue
        off = (dy - 1) * W2 + (dx - 1)
        tmp = tmps.tile([P, N2], BF16, tag="tmps")
        if dx == 1:
            nc.vector.tensor_scalar(tmp[:], X2BG[:, tq, bass.ds(GD + off, N2)],
                                    DWQW[:, tq, i:i + 1], None, ALU.mult)
        else:
            nc.scalar.activation(tmp[:], X2BG[:, tq, bass.ds(GD + off, N2)],
                                 ACTF.Copy, scale=DWQW[:, tq, i:i + 1])
        nc.vector.tensor_add(DWQ[:, tq, :], DWQ[:, tq, :], tmp[:])
    _dwq_colfix(tq)

    # tile 0: PE diagonal depthwise
    for t in range(1):
        for half in range(2):
            accs = []
            for j in range(4):
                acc = ps.tile([P, 512], FP32, tag="mm512")
                accs.append(acc)
            for i, (dy, dx) in enumerate(TAPS):
                off = (dy - 1) * W2 + (dx - 1)
                dg = dgp.tile([P, P], BF16, tag="dg")
                dma(dg[:], dram["pdiagq"].ap()[bass.ds(P * (9 * t + i), P), :])
                for j in range(4):
                    c0 = (half * 4 + j) * 512
                    nc.tensor.matmul(accs[j][:], dg[:],
                                     X2BG[:, t, bass.ds(GD + c0 + off, 512)],
                                     start=(i == 0), stop=(i == 8))
            for j in range(4):
                c0 = (half * 4 + j) * 512
                nc.scalar.copy(DWQ[:, t, bass.ds(c0, 512)], accs[j][:])
        _dwq_colfix(t)

    # ---------------- conv_ch transposed (for the residue, consumed late)
    X1CT = work.tile([P, 8, OUT_CH], BF16, tag="X1CT")
    for m in range(8):
        acc = ps.tile([P, 512], FP32, tag="mm512")
        for kk in range(4):
            nc.tensor.matmul(acc[:, 0:OUT_CH], X1B[:, kk, bass.ts(m, P)],
                             WCH[:, kk, :], start=(kk == 0), stop=(kk == 3))
        nc.scalar.copy(X1CT[:, m, :], acc[:, 0:OUT_CH])

    # ---------------- BN AllReduce
    cc1i = dpool.tile([P, 12], FP32, tag="cc1i")
    cc1o = dpool.tile([P, 12], FP32, tag="cc1o")
    dma(cc1i[:], ccin[:])
    nc.gpsimd.collective_compute("AllReduce", ALU.add,
                                 replica_groups=[list(range(NCORES))],
                                 ins=[cc1i.opt()], outs=[cc1o.opt()])
    ccout = work.tile([P, 12], FP32, tag="ccout")
    dma(ccout[:], cc1o[:])

    bnS = work.tile([P, 6], FP32, tag="bnS")
    bnT = work.tile([P, 6], FP32, tag="bnT")
    mean6 = work.tile([P, 6], FP32, tag="mean6")
    var6 = work.tile([P, 6], FP32, tag="var6")
    for t in range(6):
        n = float(B * (N1 if t < 4 else N2))
        S, S2 = ccout[:, 2 * t:2 * t + 1], ccout[:, 2 * t + 1:2 * t + 2]
        m, v = mean6[:, t:t + 1], var6[:, t:t + 1]
        nc.vector.tensor_scalar(m, S, 1.0 / n, None, ALU.mult)
        nc.vector.scalar_tensor_tensor(v, m, -1.0, m, ALU.mult, ALU.mult)
        nc.vector.scalar_tensor_tensor(v, S2, 1.0 / n, v, ALU.mult, ALU.add)
        nc.vector.tensor_scalar(v, v, EPS_BN, None, ALU.add)
    nc.vector.reciprocal(var6[:], var6[:])
    nc.scalar.activation(bnS[:], var6[:], ACTF.Sqrt)
    nc.vector.tensor_mul(bnS[:, 0:4], bnS[:, 0:4], BNPK[:, 0:4])
    nc.vector.tensor_mul(bnS[:, 4:6], bnS[:, 4:6], BNPK[:, 8:10])
    nc.vector.tensor_mul(mean6[:], mean6[:], bnS[:])
    nc.vector.tensor_sub(bnT[:, 0:4], BNPK[:, 4:8], mean6[:, 0:4])
    nc.vector.tensor_sub(bnT[:, 4:6], BNPK[:, 10:12], mean6[:, 4:6])

    # fold S into the depthwise outputs (in place, post-AllReduce);
    # U = W^T (T .* w_tap) border vectors
    TW1 = work.tile([P, 4, 9], BF16, tag="TW1")
    for t in range(4):
        nc.vector.tensor_scalar(DW1[:, t, :], DW1[:, t, :],
                                bnS[:, t:t + 1], None, ALU.mult)
        nc.vector.tensor_scalar(TW1[:, t, :], DW1W[:, t, :],
                                bnT[:, t:t + 1], None, ALU.mult)
    TWQ = work.tile([P, 2, 9], BF16, tag="TWQ")
    for t in range(2):
        nc.vector.tensor_scalar(DWQ[:, t, :], DWQ[:, t, :],
                                bnS[:, 4 + t:5 + t], None, ALU.mult)
        nc.vector.tensor_scalar(TWQ[:, t, :], DWQW[:, t, :],
                                bnT[:, 4 + t:5 + t], None, ALU.mult)
    UKV = tmps.tile([9, 2 * OUT_CH], BF16, tag="tmps")
    acc = pss.tile([P, 512], FP32, tag="psmall")
    for kk in range(4):
        nc.tensor.matmul(acc[0:9, :], TW1[:, kk, :], WKV[:, kk, :],
                         start=(kk == 0), stop=(kk == 3))
    nc.scalar.copy(UKV[:], acc[0:9, :])
    UQ = work.tile([9, OUT_CH], BF16, tag="UQ")
    acc = pss.tile([P, 512], FP32, tag="psmall")
    for kk in range(2):
        nc.tensor.matmul(acc[0:9, 0:OUT_CH], TWQ[:, kk, :], WQ[:, kk, :],
                         start=(kk == 0), stop=(kk == 1))
    nc.scalar.copy(UQ[:], acc[0:9, 0:OUT_CH])

    # ---------------- kv pointwise (pixel-major) with BN-border correction
    for m in range(8):
        acc = ps.tile([P, 512], FP32, tag="mm512")
        for kk in range(4):
            nc.tensor.matmul(acc[:], DW1[:, kk, bass.ts(m, P)], WKV[:, kk, :],
                             start=(kk == 0), stop=False)
        band = 0 if m == 0 else (2 if m == 7 else 1)
        nc.tensor.matmul(acc[:], VALK[:, band, :], UKV[:],
                         start=False, stop=True)
        nc.scalar.copy(KVT[:, m, :], acc[:])

    # resize 32->16: kvsT = RT16^T @ KVT  [256 smallpix, 512]
    KVS = []
    for mm in range(2):
        acc = pss.tile([P, 512], FP32, tag="psmall")
        for kk in range(8):
            nc.tensor.matmul(acc[:], RT16[:, kk, bass.ts(mm, P)], KVT[:, kk, :],
                             start=(kk == 0), stop=(kk == 7))
        KVS.append(acc)

    # LN-k + evac k' ; v' plain evac (bf16)
    KP = work.tile([P, 2, OUT_CH], BF16, tag="KP")
    VP = work.tile([P, 2, OUT_CH], BF16, tag="VP")
    ksum = work.tile([P, HEADS], FP32, tag="ksum")
    km = work.tile([P, HEADS], FP32, tag="km")
    krs = work.tile([P, HEADS], FP32, tag="krs")
    for mm in range(2):
        ksq = tmps.tile([P, OUT_CH], BF16, tag="tmps")
        kfp = tmps.tile([P, OUT_CH], BF16, tag="tmps")
        k_ap = KVS[mm][:, 0:OUT_CH].rearrange("p (h d) -> p h d", d=DIM_HEAD)
        nc.vector.tensor_reduce(ksum[:], k_ap, mybir.AxisListType.X, ALU.add,
                                opt_input=False)
        nc.scalar.activation(ksq[:], KVS[mm][:, 0:OUT_CH], ACTF.Square)
        nc.vector.tensor_reduce(krs[:], ksq[:].rearrange("p (h d) -> p h d",
                                                         d=DIM_HEAD),
                                mybir.AxisListType.X, ALU.add, opt_input=False)
        nc.vector.scalar_tensor_tensor(km[:], ksum[:], -1.0 / DIM_HEAD, ksum[:],
                                       ALU.mult, ALU.mult)
        nc.vector.tensor_add(krs[:], krs[:], km[:])
        nc.vector.tensor_scalar(krs[:], krs[:], DIM_HEAD * EPS_LN, None, ALU.add)
        nc.vector.reciprocal(krs[:], krs[:])
        nc.scalar.activation(krs[:], krs[:], ACTF.Sqrt, scale=float(DIM_HEAD))
        nc.vector.tensor_scalar(km[:], ksum[:], 1.0 / DIM_HEAD, None, ALU.mult)
        kb = km[:].unsqueeze(2).broadcast_to([P, HEADS, DIM_HEAD])
        rb = krs[:].unsqueeze(2).broadcast_to([P, HEADS, DIM_HEAD])
        t1 = kfp[:].rearrange("p (h d) -> p h d", d=DIM_HEAD)
        nc.vector.tensor_sub(t1, k_ap, kb)
        nc.vector.tensor_mul(t1, t1, rb)
        nc.vector.tensor_mul(kfp[:], kfp[:], GKB[:])
        nc.vector.tensor_add(KP[:, mm, :], kfp[:], BKB[:])
        nc.vector.tensor_copy(VP[:, mm, :], KVS[mm][:, OUT_CH:2 * OUT_CH])

    # A = K'^T V' / 32 : diagonal head blocks packed block-diagonal
    BD = work.tile([P, 2, P], BF16, tag="BD")
    nc.vector.memset(BD[:], 0.0)
    for mo in range(2):
        acc = pss.tile([P, 512], FP32, tag="psmall")
        for kk in range(2):
            nc.tensor.matmul(acc[:, 0:OUT_CH], KP[:, kk, bass.ts(mo, P)],
                             VP[:, kk, :], start=(kk == 0), stop=(kk == 1))
        for hh in range(4):
            h = mo * 4 + hh
            nc.scalar.activation(BD[bass.ds(32 * hh, 32), mo, bass.ds(32 * hh, 32)],
                                 acc[bass.ds(32 * hh, 32), bass.ds(32 * h, 32)],
                                 ACTF.Copy, scale=1.0 / DIM_HEAD)

    # BVCT8[xs, blk, c] = (bias_small^T v')/32, transposed + partition-aligned
    # for the attention-bias fold (sp2 = 16*blk + xs)
    BVCT8 = p8.tile([R, 16, OUT_CH], BF16, tag="p8")
    for mo2 in range(2):
        acc = pss.tile([P, 512], FP32, tag="psmall")
        for h in range(HEADS):
            for kk in range(2):
                nc.tensor.matmul(acc[:, bass.ds(32 * h, 32)],
                                 BVT[:, 2 * h + kk, bass.ts(mo2, P)],
                                 VP[:, kk, bass.ds(32 * h, 32)],
                                 start=(kk == 0), stop=(kk == 1))
        bvs = tmps.tile([P, OUT_CH], BF16, tag="tmps")
        nc.scalar.activation(bvs[:], acc[:, 0:OUT_CH],
                             ACTF.Copy, scale=1.0 / DIM_HEAD)
        for bb in range(8):
            dma(BVCT8[:, 8 * mo2 + bb, :], bvs[bass.ds(16 * bb, 16), :])

    # ---------------- q pointwise + LN-q stats
    Q = pbB.tile([P, 2, N2], BF16, tag="pbB")
    QSP = work.tile([P, 2, NS], FP32, tag="QSP")   # [(h*16+blk), (qs|q2s), 256]
    for nn in range(8):
        q2c = tmps.tile([P, 2, 512], BF16, tag="tmps")
        for mm in range(2):
            acc = ps.tile([P, 512], FP32, tag="mm512")
            for kk in range(2):
                nc.tensor.matmul(acc[:], WQ[:, kk, bass.ts(mm, P)],
                                 DWQ[:, kk, bass.ts(nn, 512)],
                                 start=(kk == 0), stop=False)
            bandq = 0 if nn == 0 else (2 if nn == 7 else 1)
            nc.tensor.matmul(acc[:], UQ[:, bass.ts(mm, P)],
                             VALQ[:, bandq, :], start=False, stop=True)
            nc.vector.tensor_copy(Q[:, mm, bass.ts(nn, 512)], acc[:])
            nc.vector.tensor_mul(q2c[:, mm, :], Q[:, mm, bass.ts(nn, 512)],
                                 Q[:, mm, bass.ts(nn, 512)])
        for s in range(2):
            sacc = pss.tile([P, 512], FP32, tag="psmall")
            for mm in range(2):
                rhs = Q[:, mm, bass.ts(nn, 512)] if s == 0 else q2c[:, mm, :]
                nc.tensor.matmul(sacc[0:HEADS, :], SELQ[:, mm, :], rhs,
                                 start=(mm == 0), stop=(mm == 1))
            # relayout rows: row 16nn + 2h + b  <->  (blk = 2nn+b, h)
            qsc = tmps.tile([HEADS, 512], FP32, tag="tmps")
            nc.vector.tensor_copy(qsc[:], sacc[0:HEADS, :])
            dma(QSP[bass.ds(16 * nn, 16), s, :],
                qsc[:].rearrange("h (b f) -> h b f", f=NS))

    # rs | mrs  (bf16, packed for the broadcast matmul)
    RSP = work.tile([P, 2, NS], BF16, tag="RSP")
    numt = tmps.tile([P, NS], FP32, tag="tmps")
    qsv, q2v = QSP[:, 0, :], QSP[:, 1, :]
    nc.vector.scalar_tensor_tensor(numt[:], qsv, -1.0 / DIM_HEAD, qsv, ALU.mult, ALU.mult)
    nc.vector.tensor_add(numt[:], numt[:], q2v)
    nc.vector.tensor_scalar(numt[:], numt[:], DIM_HEAD * EPS_LN, None, ALU.add)
    nc.vector.reciprocal(numt[:], numt[:])
    nc.scalar.activation(RSP[:, 0, :], numt[:], ACTF.Sqrt, scale=float(DIM_HEAD))
    nc.vector.scalar_tensor_tensor(RSP[:, 1, :], qsv, 1.0 / DIM_HEAD, RSP[:, 0, :],
                                   ALU.mult, ALU.mult)

    # ---------------- per-256-pixel block: broadcast g*stats, apply LN,
    # QA matmul (+BV via BVCT/XSEL fold), write padded O via scalar engine
    OPAD = pbA.tile([P, 2, N2 + 2 * GD], BF16, tag="pbA")
    for t in range(2):
        nc.gpsimd.memset(OPAD[:, t, 0:GD], 0.0)
        nc.gpsimd.memset(OPAD[:, t, GD + N2:], 0.0)
    rsp_flat = RSP[:].rearrange("p s f -> p (s f)")
    for blk in range(16):
        qpc = tmps.tile([P, 2, NS], BF16, tag="tmps")
        for mm in range(2):
            bacc = pss.tile([P, 512], FP32, tag="psmall")
            nc.tensor.matmul(bacc[:], SELB[:, blk, bass.ts(mm, P)], rsp_flat,
                             start=True, stop=True)
            q_sl = Q[:, mm, bass.ds(blk * NS, NS)]
            qp_sl = qpc[:, mm, :]
            nc.vector.tensor_mul(qp_sl, q_sl, bacc[:, 0:NS])
            nc.vector.scalar_tensor_tensor(qp_sl, bacc[:, NS:2 * NS], -1.0,
                                           qp_sl, ALU.mult, ALU.add)
            nc.vector.tensor_scalar(qp_sl, qp_sl, BQ[:, mm, :], None, ALU.add)
        for pk in range(2):
            acc = ps.tile([P, 512], FP32, tag="mm512")
            nc.tensor.matmul(acc[:, 0:NS], BD[:, pk, :], qpc[:, pk, :],
                             start=True, stop=False)
            nc.tensor.matmul(acc[:, 0:NS], BVCT8[:, blk, bass.ts(pk, P)],
                             XSEL[:], start=False, stop=True)
            nc.scalar.copy(OPAD[:, pk, bass.ds(GD + NS * blk, NS)], acc[:, 0:NS])

    # ---------------- to_out depthwise + pointwise + residue -> OSB
    DWO = pbB.tile([P, 2, N2], BF16, tag="pbB")

    def _dwo_colfix(t):
        ccol = tmps.tile([P, 2, H2], FP32, tag="tmps")
        nc.vector.memset(ccol[:], 0.0)
        oim = _img(OPAD[:, t, GD:GD + N2], W2)
        for dy in range(3):
            y0, y1 = max(0, 2 - dy), min(H2, H2 + 2 - dy)
            nc.vector.scalar_tensor_tensor(
                ccol[:, 0, y0:y1].unsqueeze(2),
                oim[:, y0 + dy - 2:y1 + dy - 2, W2 - 1:W2],
                DWOW[:, t, 3 * dy:3 * dy + 1], ccol[:, 0, y0:y1].unsqueeze(2),
                ALU.mult, ALU.add)
            z1 = H2 - dy
            nc.vector.scalar_tensor_tensor(
                ccol[:, 1, 0:z1].unsqueeze(2), oim[:, dy:z1 + dy, 0:1],
                DWOW[:, t, 3 * dy + 2:3 * dy + 3], ccol[:, 1, 0:z1].unsqueeze(2),
                ALU.mult, ALU.add)
        dwim = _img(DWO[:, t, :], W2)
        nc.vector.scalar_tensor_tensor(dwim[:, :, 0:1], ccol[:, 0, :].unsqueeze(2),
                                       -1.0, dwim[:, :, 0:1], ALU.mult, ALU.add)
        nc.vector.scalar_tensor_tensor(dwim[:, :, W2 - 1:W2],
                                       ccol[:, 1, :].unsqueeze(2),
                                       -1.0, dwim[:, :, W2 - 1:W2], ALU.mult, ALU.add)

    # tile 1: guarded-flat taps on vector+scalar (parallel with tile 0 on PE)
    t1 = 1
    i01 = TAPS.index((0, 1))
    nc.vector.tensor_scalar(DWO[:, t1, :],
                            OPAD[:, t1, bass.ds(GD - W2, N2)],
                            DWOW[:, t1, i01:i01 + 1], None, ALU.mult)
    for i, (dy, dx) in enumerate(TAPS):
        if (dy, dx) == (0, 1):
            continue
        off = (dy - 1) * W2 + (dx - 1)
        tmp = tmps.tile([P, N2], BF16, tag="tmps")
        if dx == 1:
            nc.vector.tensor_scalar(tmp[:], OPAD[:, t1, bass.ds(GD + off, N2)],
                                    DWOW[:, t1, i:i + 1], None, ALU.mult)
        else:
            nc.scalar.activation(tmp[:], OPAD[:, t1, bass.ds(GD + off, N2)],
                                 ACTF.Copy, scale=DWOW[:, t1, i:i + 1])
        nc.vector.tensor_add(DWO[:, t1, :], DWO[:, t1, :], tmp[:])
    _dwo_colfix(t1)

    # tile 0: PE diagonal depthwise
    for t in range(1):
        for half in range(2):
            accso = []
            for j in range(4):
                acc = ps.tile([P, 512], FP32, tag="mm512")
                accso.append(acc)
            for i, (dy, dx) in enumerate(TAPS):
                off = (dy - 1) * W2 + (dx - 1)
                dg = dgp.tile([P, P], BF16, tag="dg")
                dma(dg[:], dram["pdiago"].ap()[bass.ds(P * (9 * t + i), P), :])
                for j in range(4):
                    c0 = (half * 4 + j) * 512
                    nc.tensor.matmul(accso[j][:], dg[:],
                                     OPAD[:, t, bass.ds(GD + c0 + off, 512)],
                                     start=(i == 0), stop=(i == 8))
            for j in range(4):
                c0 = (half * 4 + j) * 512
                nc.scalar.copy(DWO[:, t, bass.ds(c0, 512)], accso[j][:])
        _dwo_colfix(t)
    OSB = pb32.tile([P, 2, N2], FP32, tag="pb32")
    soff = [0]
    for nn in range(8):
        soff.append(soff[-1] + len(_R64_KTILES[nn]))
    cc2s = work.tile([P, 2, 2], FP32, tag="cc2s")
    cc2i0 = dpool.tile([P, 2], FP32, tag="cc2i0")
    cc2i1 = dpool.tile([P, 2], FP32, tag="cc2i1")
    cc2o0 = dpool.tile([P, 2], FP32, tag="cc2o0")
    cc2o1 = dpool.tile([P, 2], FP32, tag="cc2o1")
    cc2i = [cc2i0, cc2i1]
    cc2o = [cc2o0, cc2o1]
    # tile 0 fully first so its AllReduce overlaps tile 1 compute
    for mm in range(2):
        for nn in range(8):
            used = _R64_KTILES[nn]
            racc = pss.tile([P, 512], FP32, tag="psmall")
            for i, kk in enumerate(used):
                nc.tensor.matmul(racc[:], X1CT[:, kk, bass.ts(mm, P)],
                                 R64C[:, soff[nn] + i, :],
                                 start=(i == 0), stop=(i == len(used) - 1))
            resc = tmps.tile([P, 512], FP32, tag="tmps")
            nc.scalar.activation(resc[:], racc[:], ACTF.Identity,
                                 bias=BNPK[:, 16 + mm:17 + mm])
            acc = ps.tile([P, 512], FP32, tag="mm512")
            for kk in range(2):
                nc.tensor.matmul(acc[:], WOUT[:, kk, bass.ts(mm, P)],
                                 DWO[:, kk, bass.ts(nn, 512)],
                                 start=(kk == 0), stop=(kk == 1))
            nc.vector.scalar_tensor_tensor(OSB[:, mm, bass.ts(nn, 512)],
                                           acc[:], 1.0, resc[:],
                                           ALU.mult, ALU.add)
            nc.vector.bn_stats(stat2[:, mm, 6 * nn:6 * nn + 6],
                               OSB[:, mm, bass.ts(nn, 512)])
        nc.vector.bn_aggr(agg2[:, mm, :],
                          stat2[:, mm, :].rearrange("p (c s) -> p c s", s=6))
        m = agg2[:, mm, 0:1]; v = agg2[:, mm, 1:2]
        S, S2 = cc2s[:, mm, 0:1], cc2s[:, mm, 1:2]
        nc.vector.tensor_scalar(S, m, float(N2), None, ALU.mult)
        nc.vector.tensor_mul(S2, m, m)
        nc.vector.tensor_add(S2, S2, v)
        nc.vector.tensor_scalar(S2, S2, float(N2), None, ALU.mult)
        dma(cc2i[mm][:], cc2s[:, mm, :])
        nc.gpsimd.collective_compute("AllReduce", ALU.add,
                                     replica_groups=[list(range(NCORES))],
                                     ins=[cc2i[mm].opt()], outs=[cc2o[mm].opt()])
    cc2r = work.tile([P, 2, 2], FP32, tag="cc2r")
    bn3S = work.tile([P, 2], FP32, tag="bn3S")
    bn3T = work.tile([P, 2], FP32, tag="bn3T")
    m3 = work.tile([P, 2], FP32, tag="m3")
    v3 = work.tile([P, 2], FP32, tag="v3")
    nB = float(B * N2)
    for t in range(2):
        dma(cc2r[:, t, :], cc2o[t][:])
        S, S2 = cc2r[:, t, 0:1], cc2r[:, t, 1:2]
        nc.vector.tensor_scalar(m3[:, t:t + 1], S, 1.0 / nB, None, ALU.mult)
        nc.vector.scalar_tensor_tensor(v3[:, t:t + 1], m3[:, t:t + 1], -1.0,
                                       m3[:, t:t + 1], ALU.mult, ALU.mult)
        nc.vector.scalar_tensor_tensor(v3[:, t:t + 1], S2, 1.0 / nB,
                                       v3[:, t:t + 1], ALU.mult, ALU.add)
        nc.vector.tensor_scalar(v3[:, t:t + 1], v3[:, t:t + 1], EPS_BN, None, ALU.add)
        nc.vector.reciprocal(v3[:, t:t + 1], v3[:, t:t + 1])
        nc.scalar.activation(bn3S[:, t:t + 1], v3[:, t:t + 1], ACTF.Sqrt)
        nc.vector.tensor_mul(bn3S[:, t:t + 1], bn3S[:, t:t + 1],
                             BNPK[:, 12 + t:13 + t])
        nc.vector.tensor_mul(m3[:, t:t + 1], m3[:, t:t + 1], bn3S[:, t:t + 1])
        nc.vector.tensor_sub(bn3T[:, t:t + 1], BNPK[:, 14 + t:15 + t],
                             m3[:, t:t + 1])

    # ---------------- relu(bn) + mlp + final residual -> out
    RELU = pbB.tile([P, 2, N2], BF16, tag="pbB")
    for t in range(2):
        nc.scalar.activation(RELU[:, t, :], OSB[:, t, :], ACTF.Relu,
                             bias=bn3T[:, t:t + 1], scale=bn3S[:, t:t + 1])
    out_ap = out_d.ap().rearrange("(t p) n -> p t n", p=P)
    for nn in range(8):
        for mm in range(2):
            acc = ps.tile([P, 512], FP32, tag="mm512")
            for kk in range(2):
                nc.tensor.matmul(acc[:], WMLP[:, kk, bass.ts(mm, P)],
                                 RELU[:, kk, bass.ts(nn, 512)],
                                 start=(kk == 0), stop=(kk == 1))
            fin = tmps.tile([P, 512], FP32, tag="tmps")
            nc.vector.tensor_add(fin[:], acc[:], OSB[:, mm, bass.ts(nn, 512)])
            dma(out_ap[:, mm, bass.ts(nn, 512)], fin[:])

    ctx.close()


def _build_program():
    nc = bacc.Bacc("TRN2", target_bir_lowering=False, debug=False,
                   num_devices=NCORES)
    dram = {}

    def din(name, shape, dt=FP32):
        dram[name] = nc.dram_tensor(name, list(shape), dt, kind="ExternalInput")

    din("x1", (IN_CH, N1), BF16); din("x2", (OUT_CH, N2))
    din("wch", (IN_CH, OUT_CH), BF16); din("wkv", (IN_CH, 2 * OUT_CH), BF16)
    din("wq", (OUT_CH, OUT_CH), BF16); din("wout", (OUT_CH, OUT_CH), BF16)
    din("wmlp", (OUT_CH, OUT_CH), BF16)
    din("dw1w", (IN_CH, 9)); din("dwqw", (OUT_CH, 9)); din("dwow", (OUT_CH, 9))
    din("rt16", (N1, NS), BF16); din("r64c", (_N_R64_SLOTS * P, 512), BF16)
    din("selq", (OUT_CH, HEADS), BF16); din("selb", (16 * P, OUT_CH), BF16)
    din("bvt", (HEADS * NS, NS), BF16)
    din("validk", (9, 3 * 4 * W1), BF16); din("validq", (9, 3 * 8 * W2), BF16)
    din("xsel", (R, NS), BF16)
    din("gkb", (P, OUT_CH), BF16); din("bkb", (P, OUT_CH), BF16)
    din("bq", (OUT_CH, 1)); din("pdiagq", (2 * 9 * P, P), BF16); din("pdiago", (2 * 9 * P, P), BF16)
    din("bnpk", (P, 18))
    out_d = nc.dram_tensor("out", [OUT_CH, N2], FP32, kind="ExternalOutput")

    with tile.TileContext(nc) as tc:
        _emit(nc, tc, dram, out_d)
    nc.compile()
    return nc


# ------------------------------------------------------------------- run layer

_CACHE = {}


def _get_program():
    if "nc" not in _CACHE:
        _CACHE["nc"] = _build_program()
    return _CACHE["nc"]


def kernel(**inputs):
    nc = _get_program()
    shared = _host_prep(inputs)
    x1 = np.ascontiguousarray(np.asarray(inputs["x1"], np.float32)
                              .reshape(B, IN_CH, N1).astype(ml_dtypes.bfloat16))
    x2 = np.ascontiguousarray(np.asarray(inputs["x2"], np.float32).reshape(B, OUT_CH, N2))
    in_maps = [dict(shared, x1=x1[b], x2=x2[b]) for b in range(B)]
    res = run_bass_kernel_spmd(nc, in_maps, core_ids=list(range(NCORES)))
    out = np.stack([np.asarray(res.results[b]["out"], np.float32)
                    .reshape(OUT_CH, H2, W2) for b in range(B)])
    return out


# revision 44
# speedup vs baseline: 1.0282x; 1.0282x over previous
"""Trainium2 Bass kernel for nn_BasicTransDecoderBlock (dense_transformer).

Strategy: data-parallel over batch B=8 across 8 NeuronCores (1 sample/core).
v2: BatchNorm is reassociated THROUGH the depthwise convs (DW(S*x+T) =
S*DW(x) + T*M with the border field M handled exactly as 9 rank-1 terms
folded into the pointwise matmul as extra contraction rows).  This lets all
depthwise tap work start immediately on the raw inputs and fully overlap
the BN AllReduce.  Taps run as tensor_scalar mult (4x mode) + tensor_tensor
add (2x mode) instead of 1x scalar_tensor_tensor; odd-dx tap multiplies go
to the scalar engine.  BN sums ride free on activation accum_out.  The
attention is softmax-free and reassociated: O = Q'(K'^T V')/d + (bias V')/d,
with the bias term folded into the attention matmul via a transposed-BV
(BVCT) extra contraction and per-head LN-q gain folded into the SELB
broadcast selector.

Self-contained: hardcodes all shapes; imports only the concourse runtime
shipped in the container.
"""
import sys
import numpy as np
import ml_dtypes

for _p in ("/opt/trn_rl_repo", "/root/.axon_site/_ro/trn_rl_repo"):
    if _p not in sys.path:
        sys.path.insert(0, _p)

import concourse.bass as bass
import concourse.bacc as bacc
import concourse.tile as tile
from concourse import mybir
from concourse.bass_utils import run_bass_kernel_spmd

FP32 = mybir.dt.float32
BF16 = mybir.dt.bfloat16
ALU = mybir.AluOpType
ACTF = mybir.ActivationFunctionType

B, IN_CH, OUT_CH, HEADS, DIM_HEAD, R = 8, 512, 256, 8, 32, 16
H1, W1, H2, W2 = 32, 32, 64, 64
EPS_BN, EPS_LN = 1e-5, 1e-6
N1, N2, NS = H1 * W1, H2 * W2, R * R     # 1024, 4096, 256
P = 128
NCORES = 8
PW1, PW2 = W1 + 2, W2 + 2                # padded widths 34, 66
PAD1, PAD2 = (H1 + 2) * PW1, (H2 + 2) * PW2   # 1156, 4356
TAPS = [(dy, dx) for dy in range(3) for dx in range(3)]


# ---------------------------------------------------------------- host helpers

def _interp_matrix(n_in, n_out):
    A = np.zeros((n_out, n_in), np.float32)
    xs = np.linspace(0.0, n_in - 1.0, n_out)
    for i, x in enumerate(xs):
        x0 = int(np.floor(x)); x1 = min(x0 + 1, n_in - 1)
        w = x - x0
        A[i, x0] += 1.0 - w
        A[i, x1] += w
    return A


def _head_major_perm():
    perm = np.zeros(OUT_CH, np.int64)
    for h in range(HEADS):
        for d in range(DIM_HEAD):
            perm[h * DIM_HEAD + d] = d * HEADS + h
    return perm


def _rel_bias_small(rel_table):
    c = np.stack(np.meshgrid(np.arange(R), np.arange(R), indexing="ij")).reshape(2, -1)
    rel = (c[:, :, None] - c[:, None, :]).transpose(1, 2, 0)
    rel[:, :, 0] += R - 1
    rel[:, :, 1] += R - 1
    rel[:, :, 0] *= 2 * R - 1
    idx = rel.sum(-1).reshape(-1)
    return np.asarray(rel_table, np.float32)[idx].reshape(NS, NS, HEADS)


def _r64_chunks():
    """Residue resize (32->64), ch-major: per 512-pixel output chunk only a
    few 128-pixel input tiles contribute."""
    Ay, Ax = _interp_matrix(H1, H2), _interp_matrix(W1, W2)
    R64 = np.kron(Ay, Ax).astype(np.float32)       # [4096, 1024]
    ktiles, blocks = [], []
    for nn in range(8):
        rows = R64[nn * 512:(nn + 1) * 512]
        used = [kk for kk in range(8)
                if np.abs(rows[:, kk * 128:(kk + 1) * 128]).sum() > 0]
        ktiles.append(used)
        for kk in used:
            blocks.append(rows[:, kk * 128:(kk + 1) * 128].T.copy())
    return ktiles, np.concatenate(blocks, axis=0)


_R64_KTILES, _R64_PACKED = _r64_chunks()
_N_R64_SLOTS = sum(len(k) for k in _R64_KTILES)


def _valid_field(H, W, rows):
    """[9, 3, rows*W] tap-validity band patterns: 0=top rows, 1=middle,
    2=bottom rows (chunks repeat the middle pattern)."""
    v = np.zeros((9, 3, rows * W), np.float32)
    for b, y0 in enumerate((0, rows, H - rows)):
        ys, xs = np.mgrid[y0:y0 + rows, 0:W]
        for i, (dy, dx) in enumerate(TAPS):
            ok = ((ys + dy - 1 >= 0) & (ys + dy - 1 < H)
                  & (xs + dx - 1 >= 0) & (xs + dx - 1 < W))
            v[i, b] = ok.reshape(-1)
    return v.reshape(9, 3 * rows * W)


def _host_prep(inp):
    perm = _head_major_perm()
    f32 = lambda a: np.ascontiguousarray(np.asarray(a, np.float32))
    bf = lambda a: np.ascontiguousarray(np.asarray(a, np.float32).astype(ml_dtypes.bfloat16))

    kvw = np.asarray(inp["to_kv_pw"], np.float32).reshape(2 * OUT_CH, IN_CH)
    gq_flat = np.asarray(inp["normq_g"], np.float32).reshape(OUT_CH)
    selb = np.zeros((16 * P, OUT_CH), np.float32)
    hh = np.arange(OUT_CH) // DIM_HEAD
    for blk in range(16):
        for h in range(HEADS):
            # stats row packing (set by the relayout DMA stream order):
            # row = 16*(blk//2) + 2h + (blk%2)
            selb[blk * P + 16 * (blk // 2) + 2 * h + (blk % 2), :] = \
                (hh == h) * gq_flat
    xsel = np.zeros((R, NS), np.float32)      # [xs, yr*64+x] = (x//4 == xs)
    for x in range(W2):
        for yr in range(4):
            xsel[x // 4, yr * W2 + x] = 1.0
    d = {
        "wch": bf(np.asarray(inp["conv_ch_w"], np.float32).reshape(OUT_CH, IN_CH).T),
        "wkv": bf(np.concatenate([kvw[perm].T, kvw[OUT_CH + perm].T], axis=1)),
        "wq": bf(np.asarray(inp["to_q_pw"], np.float32).reshape(OUT_CH, OUT_CH)[perm].T),
        "wout": bf(np.asarray(inp["to_out_pw"], np.float32).reshape(OUT_CH, OUT_CH)[:, perm].T),
        "wmlp": bf(np.asarray(inp["mlp_w"], np.float32).reshape(OUT_CH, OUT_CH).T),
        "dw1w": f32(np.asarray(inp["to_kv_dw"], np.float32).reshape(IN_CH, 9)),
        "dwqw": f32(np.asarray(inp["to_q_dw"], np.float32).reshape(OUT_CH, 9)),
        "dwow": f32(np.asarray(inp["to_out_dw"], np.float32).reshape(OUT_CH, 9)[perm]),
        "rt16": bf(np.kron(_interp_matrix(H1, R), _interp_matrix(W1, R)).T),
        "r64c": bf(_R64_PACKED),
        "selq": bf(np.equal(np.arange(OUT_CH)[:, None] // DIM_HEAD,
                            np.arange(HEADS)[None, :]).astype(np.float32)),
        "selb": bf(selb),
        "bvt": bf(_rel_bias_small(inp["rel_table"]).transpose(2, 1, 0)
                  .reshape(HEADS * NS, NS)),
        "validk": bf(_valid_field(H1, W1, 4)),     # 128-pixel chunks
        "validq": bf(_valid_field(H2, W2, 8)),     # 512-pixel chunks
        "xsel": bf(xsel),
        "gkb": bf(np.tile(np.asarray(inp["normk_g"], np.float32).reshape(1, OUT_CH), (P, 1))),
        "bkb": bf(np.tile(np.asarray(inp["normk_b"], np.float32).reshape(1, OUT_CH), (P, 1))),
        "bq": f32(np.asarray(inp["normq_b"], np.float32).reshape(OUT_CH, 1)),
    }
    dwq9 = np.asarray(inp["to_q_dw"], np.float32).reshape(OUT_CH, 9)
    pd = np.zeros((2 * 9 * P, P), np.float32)
    for t in range(2):
        for i in range(9):
            pd[(t * 9 + i) * P:(t * 9 + i + 1) * P, :] = \
                np.diag(dwq9[P * t:P * t + P, i])
    d["pdiagq"] = bf(pd)
    dwo9 = np.asarray(inp["to_out_dw"], np.float32).reshape(OUT_CH, 9)[perm]
    pdo = np.zeros((2 * 9 * P, P), np.float32)
    for t in range(2):
        for i in range(9):
            pdo[(t * 9 + i) * P:(t * 9 + i + 1) * P, :] = \
                np.diag(dwo9[P * t:P * t + P, i])
    d["pdiago"] = bf(pdo)
    pk = np.zeros((P, 18), np.float32)
    pk[:, 0:4] = np.asarray(inp["norm_l_g"], np.float32).reshape(4, P).T
    pk[:, 4:8] = np.asarray(inp["norm_l_b"], np.float32).reshape(4, P).T
    pk[:, 8:10] = np.asarray(inp["norm_h_g"], np.float32).reshape(2, P).T
    pk[:, 10:12] = np.asarray(inp["norm_h_b"], np.float32).reshape(2, P).T
    pk[:, 12:14] = np.asarray(inp["norm2_g"], np.float32).reshape(2, P).T
    pk[:, 14:16] = np.asarray(inp["norm2_b"], np.float32).reshape(2, P).T
    pk[:, 16:18] = np.asarray(inp["conv_ch_b"], np.float32).reshape(2, P).T
    d["bnpk"] = pk
    return d


# ---------------------------------------------------------------- device build

def _img(ap, w):
    return ap.rearrange("p (h w) -> p h w", w=w)


def _emit_borders(nc, xpad, Hs, pw):
    """zero the 1-px pad frame of xpad [p, (Hs+2)*pw]."""
    v = _img(xpad, pw)
    nc.gpsimd.memset(v[:, 0:1, :], 0.0)
    nc.gpsimd.memset(v[:, Hs + 1:Hs + 2, :], 0.0)
    nc.gpsimd.memset(v[:, 1:Hs + 1, 0:1], 0.0)
    nc.gpsimd.memset(v[:, 1:Hs + 1, pw - 2:pw], 0.0)


def _emit_dw(nc, tmppool, out, xpad, wvec, Hs, Ws, pw, act_odd=True):
    """depthwise 3x3 pad1 on raw input: out [p, Hs*Ws] bf16,
    xpad [p, (Hs+2)*pw] bf16, wvec [p, 9] fp32.
    tap (0,1) initializes dst on scalar engine; even-dx mults on vector
    (tensor_scalar, 4x), odd-dx mults on scalar engine; adds on vector
    (tensor_tensor, 2x)."""
    dst = _img(out, Ws)
    xv = _img(xpad, pw)
    srcs = {(dy, dx): xv[:, dy:dy + Hs, dx:dx + Ws] for dy, dx in TAPS}
    i01 = TAPS.index((0, 1))
    nc.scalar.activation(dst, srcs[(0, 1)], ACTF.Copy, scale=wvec[:, i01:i01 + 1])
    for i, (dy, dx) in enumerate(TAPS):
        if (dy, dx) == (0, 1):
            continue
        tmp = tmppool.tile([P, Hs * Ws], BF16, tag="tmps")
        tv = _img(tmp[:], Ws)
        if dx == 1 and act_odd:
            nc.scalar.activation(tv, srcs[(dy, dx)], ACTF.Copy,
                                 scale=wvec[:, i:i + 1])
        else:
            nc.vector.tensor_scalar(tv, srcs[(dy, dx)], wvec[:, i:i + 1],
                                    None, ALU.mult)
        nc.vector.tensor_add(out, out, tmp[:])


def _emit(nc, tc, dram, out_d):
    import contextlib
    ctx = contextlib.ExitStack()
    pool = lambda name, bufs, space="SBUF": ctx.enter_context(
        tc.tile_pool(name=name, bufs=bufs, space=space))

    consts = pool("consts", 1)
    work = pool("work", 1)        # unique-tag persistents (small)
    pb32 = pool("pb32", 1)        # 32KB class: X2 fp32 -> OSB fp32
    pbA = pool("pbA", 2)          # 17.4KB class: X2BP, OPAD
    pbB = pool("pbB", 2)          # 16KB class: X1fp32/scratch/DWQ/Q/DWO/RELU
    tmps = pool("tmps", 2)        # tap temporaries (8KB)
    p8 = pool("p8", 1)            # 8KB class: DW1 -> BVCT8
    dgp = pool("dgp", 2)          # streamed diag stationaries (256B)
    ps = pool("ps", 4, "PSUM")
    pss = pool("pss", 2, "PSUM")
    dpool = pool("dramp", 1, "DRAM")

    dma = nc.sync.dma_start

    # ---------------- raw inputs: plain fp32 (stats) + padded bf16 (taps)
    X1B = work.tile([P, 4, N1], BF16, tag="X1B")
    dma(X1B[:], dram["x1"].ap().rearrange("(t p) n -> p t n", p=P))
    X2F = pb32.tile([P, 2, N2], FP32, tag="pb32")
    dma(X2F[:], dram["x2"].ap().rearrange("(t p) n -> p t n", p=P))

    def load_c(name, shape, dt=FP32):
        t = consts.tile(shape, dt, tag=name)
        src = dram[name].ap()
        if len(shape) == 3:
            src = src.rearrange("(t p) n -> p t n", p=shape[0])
        dma(t[:], src)
        return t

    WCH = load_c("wch", [P, 4, OUT_CH], BF16)
    WKV = load_c("wkv", [P, 4, 2 * OUT_CH], BF16)
    WQ = load_c("wq", [P, 2, OUT_CH], BF16)
    WOUT = load_c("wout", [P, 2, OUT_CH], BF16)
    WMLP = load_c("wmlp", [P, 2, OUT_CH], BF16)
    DW1W = load_c("dw1w", [P, 4, 9])
    DWQW = load_c("dwqw", [P, 2, 9])
    DWOW = load_c("dwow", [P, 2, 9])
    RT16 = load_c("rt16", [P, 8, NS], BF16)
    R64C = load_c("r64c", [P, _N_R64_SLOTS, 512], BF16)
    SELQ = load_c("selq", [P, 2, HEADS], BF16)
    SELB = load_c("selb", [P, 16, OUT_CH], BF16)
    BVT = load_c("bvt", [P, 2 * HEADS, NS], BF16)
    VALK = load_c("validk", [9, 3, 4 * W1], BF16)
    VALQ = load_c("validq", [9, 3, 8 * W2], BF16)
    XSEL = load_c("xsel", [R, NS], BF16)
    GKB = load_c("gkb", [P, OUT_CH], BF16)
    BKB = load_c("bkb", [P, OUT_CH], BF16)
    BQ = load_c("bq", [P, 2, 1])
    BNPK = load_c("bnpk", [P, 18])

    # padded bf16 copies (scalar engine) + BN1 stats on vector bn_stats
    ccin = work.tile([P, 12], FP32, tag="ccin")
    X1BP = work.tile([P, 4, PAD1], BF16, tag="X1BP")
    for t in range(4):
        _emit_borders(nc, X1BP[:, t, :], H1, PW1)
        nc.scalar.copy(_img(X1BP[:, t, :], PW1)[:, 1:1 + H1, 1:1 + W1],
                       _img(X1B[:, t, :], W1))
    GD = 66                                     # flat guard (>= W2+1)
    X2BG = pbA.tile([P, 2, N2 + 2 * GD], BF16, tag="pbA")
    for t in range(2):
        nc.gpsimd.memset(X2BG[:, t, 0:GD], 0.0)
        nc.gpsimd.memset(X2BG[:, t, GD + N2:], 0.0)
        nc.scalar.copy(X2BG[:, t, GD:GD + N2], X2F[:, t, :])
    statA = work.tile([P, 4, 12], FP32, tag="statA")
    aggA = work.tile([P, 4, 2], FP32, tag="aggA")
    stat2 = work.tile([P, 2, 48], FP32, tag="stat2")
    agg2 = work.tile([P, 2, 2], FP32, tag="agg2")
    for t in range(4):
        for c in range(2):
            nc.vector.bn_stats(statA[:, t, 6 * c:6 * c + 6], X1B[:, t, bass.ts(c, 512)])
        nc.vector.bn_aggr(aggA[:, t, :],
                          statA[:, t, :].rearrange("p (c s) -> p c s", s=6))
    for t in range(2):
        for c in range(8):
            nc.vector.bn_stats(stat2[:, t, 6 * c:6 * c + 6], X2F[:, t, bass.ts(c, 512)])
        nc.vector.bn_aggr(agg2[:, t, :],
                          stat2[:, t, :].rearrange("p (c s) -> p c s", s=6))
    for t in range(6):
        n = float(N1 if t < 4 else N2)
        ag = aggA[:, t, :] if t < 4 else agg2[:, t - 4, :]
        m = ag[:, 0:1]; v = ag[:, 1:2]
        S, S2 = ccin[:, 2 * t:2 * t + 1], ccin[:, 2 * t + 1:2 * t + 2]
        nc.vector.tensor_scalar(S, m, n, None, ALU.mult)
        nc.vector.tensor_mul(S2, m, m)
        nc.vector.tensor_add(S2, S2, v)
        nc.vector.tensor_scalar(S2, S2, n, None, ALU.mult)

    # ---------------- depthwise on RAW inputs (overlaps the AllReduce)
    DW1 = p8.tile([P, 4, N1], BF16, tag="p8")
    for t in range(4):
        _emit_dw(nc, tmps, DW1[:, t, :], X1BP[:, t, :], DW1W[:, t, :], H1, W1, PW1)
    KVT = pbB.tile([P, 8, 2 * OUT_CH], BF16, tag="pbB")
    DWQ = pbB.tile([P, 2, N2], BF16, tag="pbB")

    def _dwq_colfix(t):
        ccol = tmps.tile([P, 2, H2], FP32, tag="tmps")
        nc.vector.memset(ccol[:], 0.0)
        xim = _img(X2F[:, t, :], W2)
        for dy in range(3):
            y0, y1 = max(0, 2 - dy), min(H2, H2 + 2 - dy)
            nc.vector.scalar_tensor_tensor(
                ccol[:, 0, y0:y1].unsqueeze(2),
                xim[:, y0 + dy - 2:y1 + dy - 2, W2 - 1:W2],
                DWQW[:, t, 3 * dy:3 * dy + 1], ccol[:, 0, y0:y1].unsqueeze(2),
                ALU.mult, ALU.add)
            z1 = H2 - dy
            nc.vector.scalar_tensor_tensor(
                ccol[:, 1, 0:z1].unsqueeze(2), xim[:, dy:z1 + dy, 0:1],
                DWQW[:, t, 3 * dy + 2:3 * dy + 3], ccol[:, 1, 0:z1].unsqueeze(2),
                ALU.mult, ALU.add)
        dwim = _img(DWQ[:, t, :], W2)
        nc.vector.scalar_tensor_tensor(dwim[:, :, 0:1], ccol[:, 0, :].unsqueeze(2),
                                       -1.0, dwim[:, :, 0:1], ALU.mult, ALU.add)
        nc.vector.scalar_tensor_tensor(dwim[:, :, W2 - 1:W2],
                                       ccol[:, 1, :].unsqueeze(2),
                                       -1.0, dwim[:, :, W2 - 1:W2], ALU.mult, ALU.add)

    # tile 1: guarded-flat taps on vector+scalar (parallel with tile 0 on PE)
    tq = 1
    iq01 = TAPS.index((0, 1))
    nc.vector.tensor_scalar(DWQ[:, tq, :],
                            X2BG[:, tq, bass.ds(GD - W2, N2)],
                            DWQW[:, tq, iq01:iq01 + 1], None, ALU.mult)
    for i, (dy, dx) in enumerate(TAPS):
        if (dy, dx) == (0, 1):
            continue
        off = (dy - 1) * W2 + (dx - 1)
        tmp = tmps.tile([P, N2], BF16, tag="tmps")
        if dx == 1:
            nc.vector.tensor_scalar(tmp[:], X2BG[:, tq, bass.ds(GD + off, N2)],
                                    DWQW[:, tq, i:i + 1], None, ALU.mult)
        else:
            nc.scalar.activation(tmp[:], X2BG[:, tq, bass.ds(GD + off, N2)],
                                 ACTF.Copy, scale=DWQW[:, tq, i:i + 1])
        nc.vector.tensor_add(DWQ[:, tq, :], DWQ[:, tq, :], tmp[:])
    _dwq_colfix(tq)

    # tile 0: PE diagonal depthwise
    for t in range(1):
        for half in range(2):
            accs = []
            for j in range(4):
                acc = ps.tile([P, 512], FP32, tag="mm512")
                accs.append(acc)
            for i, (dy, dx) in enumerate(TAPS):
                off = (dy - 1) * W2 + (dx - 1)
                dg = dgp.tile([P, P], BF16, tag="dg")
                dma(dg[:], dram["pdiagq"].ap()[bass.ds(P * (9 * t + i), P), :])
                for j in range(4):
                    c0 = (half * 4 + j) * 512
                    nc.tensor.matmul(accs[j][:], dg[:],
                                     X2BG[:, t, bass.ds(GD + c0 + off, 512)],
                                     start=(i == 0), stop=(i == 8))
            for j in range(4):
                c0 = (half * 4 + j) * 512
                nc.scalar.copy(DWQ[:, t, bass.ds(c0, 512)], accs[j][:])
        _dwq_colfix(t)

    # ---------------- BN AllReduce
    cc1i = dpool.tile([P, 12], FP32, tag="cc1i")
    cc1o = dpool.tile([P, 12], FP32, tag="cc1o")
    dma(cc1i[:], ccin[:])
    nc.gpsimd.collective_compute("AllReduce", ALU.add,
                                 replica_groups=[list(range(NCORES))],
                                 ins=[cc1i.opt()], outs=[cc1o.opt()])
    ccout = work.tile([P, 12], FP32, tag="ccout")
    dma(ccout[:], cc1o[:])

    bnS = work.tile([P, 6], FP32, tag="bnS")
    bnT = work.tile([P, 6], FP32, tag="bnT")
    mean6 = work.tile([P, 6], FP32, tag="mean6")
    var6 = work.tile([P, 6], FP32, tag="var6")
    for t in range(6):
        n = float(B * (N1 if t < 4 else N2))
        S, S2 = ccout[:, 2 * t:2 * t + 1], ccout[:, 2 * t + 1:2 * t + 2]
        m, v = mean6[:, t:t + 1], var6[:, t:t + 1]
        nc.vector.tensor_scalar(m, S, 1.0 / n, None, ALU.mult)
        nc.vector.scalar_tensor_tensor(v, m, -1.0, m, ALU.mult, ALU.mult)
        nc.vector.scalar_tensor_tensor(v, S2, 1.0 / n, v, ALU.mult, ALU.add)
        nc.vector.tensor_scalar(v, v, EPS_BN, None, ALU.add)
    nc.vector.reciprocal(var6[:], var6[:])
    nc.scalar.activation(bnS[:], var6[:], ACTF.Sqrt)
    nc.vector.tensor_mul(bnS[:, 0:4], bnS[:, 0:4], BNPK[:, 0:4])
    nc.vector.tensor_mul(bnS[:, 4:6], bnS[:, 4:6], BNPK[:, 8:10])
    nc.vector.tensor_mul(mean6[:], mean6[:], bnS[:])
    nc.vector.tensor_sub(bnT[:, 0:4], BNPK[:, 4:8], mean6[:, 0:4])
    nc.vector.tensor_sub(bnT[:, 4:6], BNPK[:, 10:12], mean6[:, 4:6])

    # fold S into the depthwise outputs (in place, post-AllReduce);
    # U = W^T (T .* w_tap) border vectors
    TW1 = work.tile([P, 4, 9], BF16, tag="TW1")
    for t in range(4):
        nc.vector.tensor_scalar(DW1[:, t, :], DW1[:, t, :],
                                bnS[:, t:t + 1], None, ALU.mult)
        nc.vector.tensor_scalar(TW1[:, t, :], DW1W[:, t, :],
                                bnT[:, t:t + 1], None, ALU.mult)
    TWQ = work.tile([P, 2, 9], BF16, tag="TWQ")
    for t in range(2):
        nc.vector.tensor_scalar(DWQ[:, t, :], DWQ[:, t, :],
                                bnS[:, 4 + t:5 + t], None, ALU.mult)
        nc.vector.tensor_scalar(TWQ[:, t, :], DWQW[:, t, :],
                                bnT[:, 4 + t:5 + t], None, ALU.mult)
    UKV = tmps.tile([9, 2 * OUT_CH], BF16, tag="tmps")
    acc = pss.tile([P, 512], FP32, tag="psmall")
    for kk in range(4):
        nc.tensor.matmul(acc[0:9, :], TW1[:, kk, :], WKV[:, kk, :],
                         start=(kk == 0), stop=(kk == 3))
    nc.scalar.copy(UKV[:], acc[0:9, :])
    UQ = work.tile([9, OUT_CH], BF16, tag="UQ")
    acc = pss.tile([P, 512], FP32, tag="psmall")
    for kk in range(2):
        nc.tensor.matmul(acc[0:9, 0:OUT_CH], TWQ[:, kk, :], WQ[:, kk, :],
                         start=(kk == 0), stop=(kk == 1))
    nc.scalar.copy(UQ[:], acc[0:9, 0:OUT_CH])

    # ---------------- conv_ch transposed (for the residue, consumed late)
    X1CT = work.tile([P, 8, OUT_CH], BF16, tag="X1CT")
    for m in range(8):
        acc = ps.tile([P, 512], FP32, tag="mm512")
        for kk in range(4):
            nc.tensor.matmul(acc[:, 0:OUT_CH], X1B[:, kk, bass.ts(m, P)],
                             WCH[:, kk, :], start=(kk == 0), stop=(kk == 3))
        nc.scalar.copy(X1CT[:, m, :], acc[:, 0:OUT_CH])

    # ---------------- kv pointwise (pixel-major) with BN-border correction
    for m in range(8):
        acc = ps.tile([P, 512], FP32, tag="mm512")
        for kk in range(4):
            nc.tensor.matmul(acc[:], DW1[:, kk, bass.ts(m, P)], WKV[:, kk, :],
                             start=(kk == 0), stop=False)
        band = 0 if m == 0 else (2 if m == 7 else 1)
        nc.tensor.matmul(acc[:], VALK[:, band, :], UKV[:],
                         start=False, stop=True)
        nc.scalar.copy(KVT[:, m, :], acc[:])

    # resize 32->16: kvsT = RT16^T @ KVT  [256 smallpix, 512]
    KVS = []
    for mm in range(2):
        acc = pss.tile([P, 512], FP32, tag="psmall")
        for kk in range(8):
            nc.tensor.matmul(acc[:], RT16[:, kk, bass.ts(mm, P)], KVT[:, kk, :],
                             start=(kk == 0), stop=(kk == 7))
        KVS.append(acc)

    # LN-k + evac k' ; v' plain evac (bf16)
    KP = work.tile([P, 2, OUT_CH], BF16, tag="KP")
    VP = work.tile([P, 2, OUT_CH], BF16, tag="VP")
    ksum = work.tile([P, HEADS], FP32, tag="ksum")
    km = work.tile([P, HEADS], FP32, tag="km")
    krs = work.tile([P, HEADS], FP32, tag="krs")
    for mm in range(2):
        ksq = tmps.tile([P, OUT_CH], BF16, tag="tmps")
        kfp = tmps.tile([P, OUT_CH], BF16, tag="tmps")
        k_ap = KVS[mm][:, 0:OUT_CH].rearrange("p (h d) -> p h d", d=DIM_HEAD)
        nc.vector.tensor_reduce(ksum[:], k_ap, mybir.AxisListType.X, ALU.add,
                                opt_input=False)
        nc.scalar.activation(ksq[:], KVS[mm][:, 0:OUT_CH], ACTF.Square)
        nc.vector.tensor_reduce(krs[:], ksq[:].rearrange("p (h d) -> p h d",
                                                         d=DIM_HEAD),
                                mybir.AxisListType.X, ALU.add, opt_input=False)
        nc.vector.scalar_tensor_tensor(km[:], ksum[:], -1.0 / DIM_HEAD, ksum[:],
                                       ALU.mult, ALU.mult)
        nc.vector.tensor_add(krs[:], krs[:], km[:])
        nc.vector.tensor_scalar(krs[:], krs[:], DIM_HEAD * EPS_LN, None, ALU.add)
        nc.vector.reciprocal(krs[:], krs[:])
        nc.scalar.activation(krs[:], krs[:], ACTF.Sqrt, scale=float(DIM_HEAD))
        nc.vector.tensor_scalar(km[:], ksum[:], 1.0 / DIM_HEAD, None, ALU.mult)
        kb = km[:].unsqueeze(2).broadcast_to([P, HEADS, DIM_HEAD])
        rb = krs[:].unsqueeze(2).broadcast_to([P, HEADS, DIM_HEAD])
        t1 = kfp[:].rearrange("p (h d) -> p h d", d=DIM_HEAD)
        nc.vector.tensor_sub(t1, k_ap, kb)
        nc.vector.tensor_mul(t1, t1, rb)
        nc.vector.tensor_mul(kfp[:], kfp[:], GKB[:])
        nc.vector.tensor_add(KP[:, mm, :], kfp[:], BKB[:])
        nc.vector.tensor_copy(VP[:, mm, :], KVS[mm][:, OUT_CH:2 * OUT_CH])

    # A = K'^T V' / 32 : diagonal head blocks packed block-diagonal
    BD = work.tile([P, 2, P], BF16, tag="BD")
    nc.vector.memset(BD[:], 0.0)
    for mo in range(2):
        acc = pss.tile([P, 512], FP32, tag="psmall")
        for kk in range(2):
            nc.tensor.matmul(acc[:, 0:OUT_CH], KP[:, kk, bass.ts(mo, P)],
                             VP[:, kk, :], start=(kk == 0), stop=(kk == 1))
        for hh in range(4):
            h = mo * 4 + hh
            nc.scalar.activation(BD[bass.ds(32 * hh, 32), mo, bass.ds(32 * hh, 32)],
                                 acc[bass.ds(32 * hh, 32), bass.ds(32 * h, 32)],
                                 ACTF.Copy, scale=1.0 / DIM_HEAD)

    # BVCT8[xs, blk, c] = (bias_small^T v')/32, transposed + partition-aligned
    # for the attention-bias fold (sp2 = 16*blk + xs)
    BVCT8 = p8.tile([R, 16, OUT_CH], BF16, tag="p8")
    for mo2 in range(2):
        acc = pss.tile([P, 512], FP32, tag="psmall")
        for h in range(HEADS):
            for kk in range(2):
                nc.tensor.matmul(acc[:, bass.ds(32 * h, 32)],
                                 BVT[:, 2 * h + kk, bass.ts(mo2, P)],
                                 VP[:, kk, bass.ds(32 * h, 32)],
                                 start=(kk == 0), stop=(kk == 1))
        bvs = tmps.tile([P, OUT_CH], BF16, tag="tmps")
        nc.scalar.activation(bvs[:], acc[:, 0:OUT_CH],
                             ACTF.Copy, scale=1.0 / DIM_HEAD)
        for bb in range(8):
            dma(BVCT8[:, 8 * mo2 + bb, :], bvs[bass.ds(16 * bb, 16), :])

    # ---------------- q pointwise + LN-q stats
    Q = pbB.tile([P, 2, N2], BF16, tag="pbB")
    QSP = work.tile([P, 2, NS], FP32, tag="QSP")   # [(h*16+blk), (qs|q2s), 256]
    for nn in range(8):
        q2c = tmps.tile([P, 2, 512], BF16, tag="tmps")
        for mm in range(2):
            acc = ps.tile([P, 512], FP32, tag="mm512")
            for kk in range(2):
                nc.tensor.matmul(acc[:], WQ[:, kk, bass.ts(mm, P)],
                                 DWQ[:, kk, bass.ts(nn, 512)],
                                 start=(kk == 0), stop=False)
            bandq = 0 if nn == 0 else (2 if nn == 7 else 1)
            nc.tensor.matmul(acc[:], UQ[:, bass.ts(mm, P)],
                             VALQ[:, bandq, :], start=False, stop=True)
            nc.vector.tensor_copy(Q[:, mm, bass.ts(nn, 512)], acc[:])
            nc.vector.tensor_mul(q2c[:, mm, :], Q[:, mm, bass.ts(nn, 512)],
                                 Q[:, mm, bass.ts(nn, 512)])
        for s in range(2):
            sacc = pss.tile([P, 512], FP32, tag="psmall")
            for mm in range(2):
                rhs = Q[:, mm, bass.ts(nn, 512)] if s == 0 else q2c[:, mm, :]
                nc.tensor.matmul(sacc[0:HEADS, :], SELQ[:, mm, :], rhs,
                                 start=(mm == 0), stop=(mm == 1))
            # relayout rows: row 16nn + 2h + b  <->  (blk = 2nn+b, h)
            qsc = tmps.tile([HEADS, 512], FP32, tag="tmps")
            nc.vector.tensor_copy(qsc[:], sacc[0:HEADS, :])
            dma(QSP[bass.ds(16 * nn, 16), s, :],
                qsc[:].rearrange("h (b f) -> h b f", f=NS))

    # rs | mrs  (bf16, packed for the broadcast matmul)
    RSP = work.tile([P, 2, NS], BF16, tag="RSP")
    numt = tmps.tile([P, NS], FP32, tag="tmps")
    qsv, q2v = QSP[:, 0, :], QSP[:, 1, :]
    nc.vector.scalar_tensor_tensor(numt[:], qsv, -1.0 / DIM_HEAD, qsv, ALU.mult, ALU.mult)
    nc.vector.tensor_add(numt[:], numt[:], q2v)
    nc.vector.tensor_scalar(numt[:], numt[:], DIM_HEAD * EPS_LN, None, ALU.add)
    nc.vector.reciprocal(numt[:], numt[:])
    nc.scalar.activation(RSP[:, 0, :], numt[:], ACTF.Sqrt, scale=float(DIM_HEAD))
    nc.vector.scalar_tensor_tensor(RSP[:, 1, :], qsv, 1.0 / DIM_HEAD, RSP[:, 0, :],
                                   ALU.mult, ALU.mult)

    # ---------------- per-256-pixel block: broadcast g*stats, apply LN,
    # QA matmul (+BV via BVCT/XSEL fold), write padded O via scalar engine
    OPAD = pbA.tile([P, 2, N2 + 2 * GD], BF16, tag="pbA")
    for t in range(2):
        nc.gpsimd.memset(OPAD[:, t, 0:GD], 0.0)
        nc.gpsimd.memset(OPAD[:, t, GD + N2:], 0.0)
    rsp_flat = RSP[:].rearrange("p s f -> p (s f)")
    for blk in range(16):
        qpc = tmps.tile([P, 2, NS], BF16, tag="tmps")
        for mm in range(2):
            bacc = pss.tile([P, 512], FP32, tag="psmall")
            nc.tensor.matmul(bacc[:], SELB[:, blk, bass.ts(mm, P)], rsp_flat,
                             start=True, stop=True)
            q_sl = Q[:, mm, bass.ds(blk * NS, NS)]
            qp_sl = qpc[:, mm, :]
            nc.vector.tensor_mul(qp_sl, q_sl, bacc[:, 0:NS])
            nc.vector.scalar_tensor_tensor(qp_sl, bacc[:, NS:2 * NS], -1.0,
                                           qp_sl, ALU.mult, ALU.add)
            nc.vector.tensor_scalar(qp_sl, qp_sl, BQ[:, mm, :], None, ALU.add)
        for pk in range(2):
            acc = ps.tile([P, 512], FP32, tag="mm512")
            nc.tensor.matmul(acc[:, 0:NS], BD[:, pk, :], qpc[:, pk, :],
                             start=True, stop=False)
            nc.tensor.matmul(acc[:, 0:NS], BVCT8[:, blk, bass.ts(pk, P)],
                             XSEL[:], start=False, stop=True)
            nc.scalar.copy(OPAD[:, pk, bass.ds(GD + NS * blk, NS)], acc[:, 0:NS])

    # ---------------- to_out depthwise + pointwise + residue -> OSB
    DWO = pbB.tile([P, 2, N2], BF16, tag="pbB")

    def _dwo_colfix(t):
        ccol = tmps.tile([P, 2, H2], FP32, tag="tmps")
        nc.vector.memset(ccol[:], 0.0)
        oim = _img(OPAD[:, t, GD:GD + N2], W2)
        for dy in range(3):
            y0, y1 = max(0, 2 - dy), min(H2, H2 + 2 - dy)
            nc.vector.scalar_tensor_tensor(
                ccol[:, 0, y0:y1].unsqueeze(2),
                oim[:, y0 + dy - 2:y1 + dy - 2, W2 - 1:W2],
                DWOW[:, t, 3 * dy:3 * dy + 1], ccol[:, 0, y0:y1].unsqueeze(2),
                ALU.mult, ALU.add)
            z1 = H2 - dy
            nc.vector.scalar_tensor_tensor(
                ccol[:, 1, 0:z1].unsqueeze(2), oim[:, dy:z1 + dy, 0:1],
                DWOW[:, t, 3 * dy + 2:3 * dy + 3], ccol[:, 1, 0:z1].unsqueeze(2),
                ALU.mult, ALU.add)
        dwim = _img(DWO[:, t, :], W2)
        nc.vector.scalar_tensor_tensor(dwim[:, :, 0:1], ccol[:, 0, :].unsqueeze(2),
                                       -1.0, dwim[:, :, 0:1], ALU.mult, ALU.add)
        nc.vector.scalar_tensor_tensor(dwim[:, :, W2 - 1:W2],
                                       ccol[:, 1, :].unsqueeze(2),
                                       -1.0, dwim[:, :, W2 - 1:W2], ALU.mult, ALU.add)

    # tile 1: guarded-flat taps on vector+scalar (parallel with tile 0 on PE)
    t1 = 1
    i01 = TAPS.index((0, 1))
    nc.vector.tensor_scalar(DWO[:, t1, :],
                            OPAD[:, t1, bass.ds(GD - W2, N2)],
                            DWOW[:, t1, i01:i01 + 1], None, ALU.mult)
    for i, (dy, dx) in enumerate(TAPS):
        if (dy, dx) == (0, 1):
            continue
        off = (dy - 1) * W2 + (dx - 1)
        tmp = tmps.tile([P, N2], BF16, tag="tmps")
        if dx == 1:
            nc.vector.tensor_scalar(tmp[:], OPAD[:, t1, bass.ds(GD + off, N2)],
                                    DWOW[:, t1, i:i + 1], None, ALU.mult)
        else:
            nc.scalar.activation(tmp[:], OPAD[:, t1, bass.ds(GD + off, N2)],
                                 ACTF.Copy, scale=DWOW[:, t1, i:i + 1])
        nc.vector.tensor_add(DWO[:, t1, :], DWO[:, t1, :], tmp[:])
    _dwo_colfix(t1)

    # tile 0: PE diagonal depthwise
    for t in range(1):
        for half in range(2):
            accso = []
            for j in range(4):
                acc = ps.tile([P, 512], FP32, tag="mm512")
                accso.append(acc)
            for i, (dy, dx) in enumerate(TAPS):
                off = (dy - 1) * W2 + (dx - 1)
                dg = dgp.tile([P, P], BF16, tag="dg")
                dma(dg[:], dram["pdiago"].ap()[bass.ds(P * (9 * t + i), P), :])
                for j in range(4):
                    c0 = (half * 4 + j) * 512
                    nc.tensor.matmul(accso[j][:], dg[:],
                                     OPAD[:, t, bass.ds(GD + c0 + off, 512)],
                                     start=(i == 0), stop=(i == 8))
            for j in range(4):
                c0 = (half * 4 + j) * 512
                nc.scalar.copy(DWO[:, t, bass.ds(c0, 512)], accso[j][:])
        _dwo_colfix(t)
    OSB = pb32.tile([P, 2, N2], FP32, tag="pb32")
    soff = [0]
    for nn in range(8):
        soff.append(soff[-1] + len(_R64_KTILES[nn]))
    cc2s = work.tile([P, 2, 2], FP32, tag="cc2s")
    cc2i0 = dpool.tile([P, 2], FP32, tag="cc2i0")
    cc2i1 = dpool.tile([P, 2], FP32, tag="cc2i1")
    cc2o0 = dpool.tile([P, 2], FP32, tag="cc2o0")
    cc2o1 = dpool.tile([P, 2], FP32, tag="cc2o1")
    cc2i = [cc2i0, cc2i1]
    cc2o = [cc2o0, cc2o1]
    # tile 0 fully first so its AllReduce overlaps tile 1 compute
    for mm in range(2):
        for nn in range(8):
            used = _R64_KTILES[nn]
            racc = pss.tile([P, 512], FP32, tag="psmall")
            for i, kk in enumerate(used):
                nc.tensor.matmul(racc[:], X1CT[:, kk, bass.ts(mm, P)],
                                 R64C[:, soff[nn] + i, :],
                                 start=(i == 0), stop=(i == len(used) - 1))
            resc = tmps.tile([P, 512], FP32, tag="tmps")
            nc.scalar.activation(resc[:], racc[:], ACTF.Identity,
                                 bias=BNPK[:, 16 + mm:17 + mm])
            acc = ps.tile([P, 512], FP32, tag="mm512")
            for kk in range(2):
                nc.tensor.matmul(acc[:], WOUT[:, kk, bass.ts(mm, P)],
                                 DWO[:, kk, bass.ts(nn, 512)],
                                 start=(kk == 0), stop=(kk == 1))
            nc.vector.scalar_tensor_tensor(OSB[:, mm, bass.ts(nn, 512)],
                                           acc[:], 1.0, resc[:],
                                           ALU.mult, ALU.add)
            nc.vector.bn_stats(stat2[:, mm, 6 * nn:6 * nn + 6],
                               OSB[:, mm, bass.ts(nn, 512)])
        nc.vector.bn_aggr(agg2[:, mm, :],
                          stat2[:, mm, :].rearrange("p (c s) -> p c s", s=6))
        m = agg2[:, mm, 0:1]; v = agg2[:, mm, 1:2]
        S, S2 = cc2s[:, mm, 0:1], cc2s[:, mm, 1:2]
        nc.vector.tensor_scalar(S, m, float(N2), None, ALU.mult)
        nc.vector.tensor_mul(S2, m, m)
        nc.vector.tensor_add(S2, S2, v)
        nc.vector.tensor_scalar(S2, S2, float(N2), None, ALU.mult)
        dma(cc2i[mm][:], cc2s[:, mm, :])
        nc.gpsimd.collective_compute("AllReduce", ALU.add,
                                     replica_groups=[list(range(NCORES))],
                                     ins=[cc2i[mm].opt()], outs=[cc2o[mm].opt()])
    cc2r = work.tile([P, 2, 2], FP32, tag="cc2r")
    bn3S = work.tile([P, 2], FP32, tag="bn3S")
    bn3T = work.tile([P, 2], FP32, tag="bn3T")
    m3 = work.tile([P, 2], FP32, tag="m3")
    v3 = work.tile([P, 2], FP32, tag="v3")
    nB = float(B * N2)
    for t in range(2):
        dma(cc2r[:, t, :], cc2o[t][:])
        S, S2 = cc2r[:, t, 0:1], cc2r[:, t, 1:2]
        nc.vector.tensor_scalar(m3[:, t:t + 1], S, 1.0 / nB, None, ALU.mult)
        nc.vector.scalar_tensor_tensor(v3[:, t:t + 1], m3[:, t:t + 1], -1.0,
                                       m3[:, t:t + 1], ALU.mult, ALU.mult)
        nc.vector.scalar_tensor_tensor(v3[:, t:t + 1], S2, 1.0 / nB,
                                       v3[:, t:t + 1], ALU.mult, ALU.add)
        nc.vector.tensor_scalar(v3[:, t:t + 1], v3[:, t:t + 1], EPS_BN, None, ALU.add)
        nc.vector.reciprocal(v3[:, t:t + 1], v3[:, t:t + 1])
        nc.scalar.activation(bn3S[:, t:t + 1], v3[:, t:t + 1], ACTF.Sqrt)
        nc.vector.tensor_mul(bn3S[:, t:t + 1], bn3S[:, t:t + 1],
                             BNPK[:, 12 + t:13 + t])
        nc.vector.tensor_mul(m3[:, t:t + 1], m3[:, t:t + 1], bn3S[:, t:t + 1])
        nc.vector.tensor_sub(bn3T[:, t:t + 1], BNPK[:, 14 + t:15 + t],
                             m3[:, t:t + 1])

    # ---------------- relu(bn) + mlp + final residual -> out
    RELU = pbB.tile([P, 2, N2], BF16, tag="pbB")
    for t in range(2):
        nc.scalar.activation(RELU[:, t, :], OSB[:, t, :], ACTF.Relu,
                             bias=bn3T[:, t:t + 1], scale=bn3S[:, t:t + 1])
    out_ap = out_d.ap().rearrange("(t p) n -> p t n", p=P)
    for nn in range(8):
        for mm in range(2):
            acc = ps.tile([P, 512], FP32, tag="mm512")
            for kk in range(2):
                nc.tensor.matmul(acc[:], WMLP[:, kk, bass.ts(mm, P)],
                                 RELU[:, kk, bass.ts(nn, 512)],
                                 start=(kk == 0), stop=(kk == 1))
            fin = tmps.tile([P, 512], FP32, tag="tmps")
            nc.vector.tensor_add(fin[:], acc[:], OSB[:, mm, bass.ts(nn, 512)])
            dma(out_ap[:, mm, bass.ts(nn, 512)], fin[:])

    ctx.close()


def _build_program():
    nc = bacc.Bacc("TRN2", target_bir_lowering=False, debug=False,
                   num_devices=NCORES)
    dram = {}

    def din(name, shape, dt=FP32):
        dram[name] = nc.dram_tensor(name, list(shape), dt, kind="ExternalInput")

    din("x1", (IN_CH, N1), BF16); din("x2", (OUT_CH, N2))
    din("wch", (IN_CH, OUT_CH), BF16); din("wkv", (IN_CH, 2 * OUT_CH), BF16)
    din("wq", (OUT_CH, OUT_CH), BF16); din("wout", (OUT_CH, OUT_CH), BF16)
    din("wmlp", (OUT_CH, OUT_CH), BF16)
    din("dw1w", (IN_CH, 9)); din("dwqw", (OUT_CH, 9)); din("dwow", (OUT_CH, 9))
    din("rt16", (N1, NS), BF16); din("r64c", (_N_R64_SLOTS * P, 512), BF16)
    din("selq", (OUT_CH, HEADS), BF16); din("selb", (16 * P, OUT_CH), BF16)
    din("bvt", (HEADS * NS, NS), BF16)
    din("validk", (9, 3 * 4 * W1), BF16); din("validq", (9, 3 * 8 * W2), BF16)
    din("xsel", (R, NS), BF16)
    din("gkb", (P, OUT_CH), BF16); din("bkb", (P, OUT_CH), BF16)
    din("bq", (OUT_CH, 1)); din("pdiagq", (2 * 9 * P, P), BF16); din("pdiago", (2 * 9 * P, P), BF16)
    din("bnpk", (P, 18))
    out_d = nc.dram_tensor("out", [OUT_CH, N2], FP32, kind="ExternalOutput")

    with tile.TileContext(nc) as tc:
        _emit(nc, tc, dram, out_d)
    nc.compile()
    return nc


# ------------------------------------------------------------------- run layer

_CACHE = {}


def _get_program():
    if "nc" not in _CACHE:
        _CACHE["nc"] = _build_program()
    return _CACHE["nc"]


def kernel(**inputs):
    nc = _get_program()
    shared = _host_prep(inputs)
    x1 = np.ascontiguousarray(np.asarray(inputs["x1"], np.float32)
                              .reshape(B, IN_CH, N1).astype(ml_dtypes.bfloat16))
    x2 = np.ascontiguousarray(np.asarray(inputs["x2"], np.float32).reshape(B, OUT_CH, N2))
    in_maps = [dict(shared, x1=x1[b], x2=x2[b]) for b in range(B)]
    res = run_bass_kernel_spmd(nc, in_maps, core_ids=list(range(NCORES)))
    out = np.stack([np.asarray(res.results[b]["out"], np.float32)
                    .reshape(OUT_CH, H2, W2) for b in range(B)])
    return out
